# revision 55
# baseline (speedup 1.0000x reference)
"""Multi-head attention (B=4, S=2048, D=1024, H=16, dk=dv=64) on 8 TRN2 NeuronCores.

Sharding: batch x head-half. Core c handles batch b = c//2 and heads
hh*8..hh*8+8 where hh = c%2. Each core computes its 8 heads' attention plus
the partial output projection (row-parallel fc); the host sums the two
partials per batch and adds the output bias.

Device algorithm per core (matmul inputs bf16, PSUM accumulation fp32):
  - inputs pre-transposed on host: xT = x.T (D on partitions) so every matmul
    contracts over the partition dimension with zero on-device transposes.
  - Q^T, K^T = W X^T   laid out [feat, seq]; K^T zero-padded per head to a
    full 128-partition stripe (keeps score matmuls K=128 and the PE activity
    monitor from clock-gating the array).
  - V = X W^T          [seq, feat] with a ones column per head so the softmax
    denominator falls out of the PV matmul.
  - S^T = K^T' Q^T per head, exp on ScalarE reading PSUM (no max-subtraction:
    |scores|/8 <~ 12, safe in fp32/bf16).
  - C^T_aug = V_aug^T expS^T accumulated over kpos; row 64 = softmax sums.
  - normalize via reciprocal; partition broadcast of 1/l through a DRAM
    bounce mid-stream (off the PE critical path) or a K=1 PE matmul at the
    tail (PE idle there).
  - out_partial = C^T.T @ Wo_c^T, fp32 to DRAM.

Scheduling (the point of this version): the kernel is Tensor-engine bound
(1536 matmuls x ~213ns = 327us at full clock), with ScalarE's exp stream
(1016ns per 128x1024 tile, 260us total) second. Emission order is produced
by a small clock simulator so the PE queue stays gapless: projection and
output-projection matmuls are interleaved as single-matmul wedges inside the
attention stream, and the scores stream runs up to BANK_CAP units ahead of
the PV stream (es backlog) so ScalarE never starves PV even in stretches
where exp/unit exceeds PE/unit. Pad memsets run on GpSimd (Pool), x-input
DMAs on the gpsimd queue (parallel DGE setup with the weight DMAs on sync),
and the first wk/xk chunks are split so the first matmul starts ~2.5us in.
PSUM: 2x2-bank score tiles double-buffer against exp; 4x1-bank accumulators
hold the two PV accumulators plus in-flight wedge chains, with per-head
alignment padding so consecutive heads' PV accumulators land on alternating
bank pairs (no drain-wait at head handoff).
"""

import sys

if "/opt/trn_rl_repo" not in sys.path:
    sys.path.insert(0, "/opt/trn_rl_repo")

from contextlib import ExitStack

import ml_dtypes
import numpy as np

import concourse.bass as bass
import concourse.tile as tile
from concourse import bacc, mybir
from concourse.bass_utils import run_bass_kernel_spmd

BF16 = mybir.dt.bfloat16
F32 = mybir.dt.float32
P = 128

B, S, D = 4, 2048, 1024
H, DH = 16, 64
G = 512          # head-group width per core: 8 heads x 64
NH = G // DH     # 8 heads per core
DC = D // P      # contraction chunks over model dim
FC = G // P      # feat chunks of the head-group width
SCALE = 1.0 / 8.0  # 1/sqrt(dk)

# emitter cost model (ns) — only relative sizes matter
MM_NS = 216.0     # one 512-row bf16 matmul at 2.4 GHz
EXP_NS = 1090.0   # one [128,1024] exp on ScalarE as observed when the PE
                  # runs flat-out (the PE at full clock slows ACT ~10%)
# S(k) is emitted only once exp(k-2) finished this long ago (sim time): the
# psA-free semaphore is then already up when the PE reaches S(k), instead of
# the PE racing ScalarE to it (a ~100ns stall + p-state throttle per unit).
PSA_SLACK = -300.0
BANK_CAP = 25     # max scores-ahead units (es backlog) — as large as SBUF
                  # allows, so ScalarE stays saturated through the slow
                  # V-projection units of the first head
DEBUG_SCHED = False
SW = 72           # V stripe width per head: 64 V cols + 1 ones col + 7 pad
                  # (was 128 with 63 zero cols — reclaimed for es buffers)


def _emit(ctx, tc, io, seq):
    nc = tc.nc
    KC = seq // P                 # key chunks (16)
    QW = 1024                     # q width per score tile (2 PSUM banks)
    NI = QW // 512                # matmul chunks per score tile (2)
    NQT = seq // QW               # q tiles (2)
    NQC = seq // 512              # 512-wide projection chunks (4)
    EXP = mybir.ActivationFunctionType.Exp
    HEAD_ORDER = [0, 1, 2, 3, 4, 5, 7, 6]  # end tiles on an even head (short
                                           # normalize chain at the tail)

    wpool = ctx.enter_context(tc.tile_pool(name="w", bufs=1))
    xpool = ctx.enter_context(tc.tile_pool(name="x", bufs=2))
    perm = ctx.enter_context(tc.tile_pool(name="perm", bufs=1))
    epool = ctx.enter_context(tc.tile_pool(name="e", bufs=BANK_CAP + 2))
    small = ctx.enter_context(tc.tile_pool(name="small", bufs=2))
    opool = ctx.enter_context(tc.tile_pool(name="o", bufs=4))
    dpool = ctx.enter_context(tc.tile_pool(name="d", bufs=3, space="DRAM"))
    psA = ctx.enter_context(tc.tile_pool(name="psA", bufs=2, space="PSUM"))
    # four explicitly-managed single-bank accumulator pools: per head, one
    # pair holds the two PV accumulators while the other pair serves wedge
    # chains; pairs swap every head (no drain-wait at head handoff).
    acc_pools = [
        ctx.enter_context(tc.tile_pool(name=f"ps{i}", bufs=1, space="PSUM"))
        for i in range(4)]

    # --- persistent weights / biases ---
    wq_sb = wpool.tile([P, DC, G], BF16, name="wq_sb")
    wk_sb = wpool.tile([P, DC, G], BF16, name="wk_sb")
    wv_sb = wpool.tile([P, DC, G], BF16, name="wv_sb")
    wo_sb = wpool.tile([P, FC, D], BF16, name="wo_sb")
    bq_sb = wpool.tile([P, FC], F32, name="bq_sb")
    bk_sb = wpool.tile([P, FC], F32, name="bk_sb")
    bv_sb = wpool.tile([P, G], F32, name="bv_sb")

    # --- persistent activations ---
    QT = perm.tile([P, FC, seq], BF16, name="QT")
    KT = perm.tile([P, NH, seq], BF16, name="KT")
    V = perm.tile([P, KC, NH * SW], BF16, name="V")
    CT = perm.tile([P, FC, seq], BF16, name="CT")
    V4 = V.rearrange("p kc (h c) -> p kc h c", h=NH)
    ones_bf = wpool.tile([P, DH], BF16, name="ones_bf")

    xk, xq, xv = {}, {}, {}

    def x_dma(store, tname, qc, split=False):
        t = xpool.tile([P, DC, 512], BF16, name=f"x{tname}{qc}", tag="xt")
        src = io["x" + tname + "T"][qc].rearrange("p (dc s) -> p dc s", s=512)
        if split:
            nc.gpsimd.dma_start(t[:, 0:2], src[:, 0:2])
            nc.gpsimd.dma_start(t[:, 2:5], src[:, 2:5])
            nc.gpsimd.dma_start(t[:, 5:], src[:, 5:])
        else:
            nc.gpsimd.dma_start(t[:], src)
        store[qc] = t

    # --- startup DMAs: weights on sync, x inputs on gpsimd (parallel DGE).
    # wk/xk0 split so their first dc chunks land first and the first matmul
    # starts ~2.5us in.
    nc.sync.dma_start(wk_sb[:, 0:2], io["wkT"][:, 0:2])
    x_dma(xk, "k", 0, split=True)
    nc.sync.dma_start(wk_sb[:, 2:5], io["wkT"][:, 2:5])
    nc.sync.dma_start(wk_sb[:, 5:], io["wkT"][:, 5:])
    nc.sync.dma_start(bk_sb[:], io["bkc"].rearrange("(fc p) -> p fc", p=P))
    x_dma(xk, "k", 1)
    # (xk2/xk3/xq/xv triggers are staggered through the prologue below so the
    # early transfers get full HBM bandwidth)

    # pad memsets on gpsimd (Pool engine — otherwise idle). They are split
    # around the staggered x DMA triggers below: the Pool queue is in-order,
    # so each trigger must come before the next memset block to fire on time.
    def kt_memset(h):
        pp = (1 - h % 2) * DH
        nc.gpsimd.memset(KT[pp:pp + DH, h:h + 1, :], 0.0)

    for h in range(4):
        kt_memset(h)

    # --- PSUM accumulator allocation ---
    rr = [0]  # round-robin cursor for prologue/tail allocations

    def acc_tile(pool_i, name):
        return acc_pools[pool_i].tile([P, 512], F32, name=name, tag="acc")

    def rr_tile(name):
        t = acc_tile(rr[0] % 4, name)
        rr[0] += 1
        return t

    # --- building blocks ---
    def proj_chain_mms(tname, qc, fc, ps):
        """The 8 accumulation matmuls for one [128,512] projection output."""
        xt = {"q": xq, "k": xk, "v": xv}[tname][qc]
        if tname == "v":
            s4 = fc  # for V, "fc" indexes the 128-seq subchunk instead
            return [
                (lambda dc=dc: nc.tensor.matmul(
                    ps[:], xt[:, dc, s4 * P:(s4 + 1) * P], wv_sb[:, dc, :],
                    start=(dc == 0), stop=(dc == DC - 1)))
                for dc in range(DC)]
        wsb = wq_sb if tname == "q" else wk_sb
        return [
            (lambda dc=dc: nc.tensor.matmul(
                ps[:], wsb[:, dc, fc * P:(fc + 1) * P], xt[:, dc, :],
                start=(dc == 0), stop=(dc == DC - 1)))
            for dc in range(DC)]

    def proj_drain(tname, qc, fc, ps):
        if tname == "q":
            nc.vector.tensor_scalar_add(
                out=QT[:, fc, qc * 512:(qc + 1) * 512], in0=ps[:],
                scalar1=bq_sb[:, fc:fc + 1])
        elif tname == "k":
            for hp in range(2):
                pp = hp * DH
                nc.vector.tensor_scalar_add(
                    out=KT[pp:pp + DH, 2 * fc + hp, qc * 512:(qc + 1) * 512],
                    in0=ps[pp:pp + DH, :], scalar1=bk_sb[pp:pp + DH, fc:fc + 1])
        else:
            kc = qc * 4 + fc
            nc.vector.tensor_add(
                out=V[:, kc].rearrange("p (h c) -> p h c", h=NH)[:, :, 0:DH],
                in0=ps.rearrange("p (h c) -> p h c", h=NH),
                in1=bv_sb.rearrange("p (h c) -> p h c", h=NH))

    def outproj_chain(qt, s8, oc, mk_tile, copy_eng=None, dma_eng=None):
        sc = qt * (QW // P) + s8
        ops = mk_tile(f"op{sc}o{oc}")
        mms = [
            (lambda fc=fc: nc.tensor.matmul(
                ops[:], CT[:, fc, sc * P:(sc + 1) * P],
                wo_sb[:, fc, oc * 512:(oc + 1) * 512],
                start=(fc == 0), stop=(fc == FC - 1)))
            for fc in range(FC)]

        def drain():
            osb = opool.tile([P, 512], BF16, name=f"ob{sc}o{oc}", tag="ob")
            if copy_eng == "scalar":
                nc.scalar.copy(out=osb[:], in_=ops[:])
            else:
                nc.vector.tensor_copy(out=osb[:], in_=ops[:])
            (dma_eng or nc.sync).dma_start(
                io["out"][sc * P:(sc + 1) * P, oc * 512:(oc + 1) * 512], osb[:])

        return mms, drain

    def emit_normalize(qt, h, cps, tail=False):
        p0 = (h % 2) * DH
        fcH = h // 2
        for i in range(NI):
            q0 = qt * QW + i * 512
            l1 = small.tile([P, 512], BF16, name=f"l{qt}h{h}i{i}", tag="l1")
            nc.vector.tensor_copy(out=l1[DH:DH + 1, :], in_=cps[i][DH:DH + 1, :])
            if tail:
                # PE is idle at the tail: broadcast 1/l via a K=1 matmul
                # instead of the higher-latency DRAM bounce.
                assert p0 == 0  # tail head must be even
                bb = acc_tile(i, f"bb{qt}h{h}i{i}")  # ex-wedge pools, free now
                nc.tensor.matmul(bb[0:DH, :], ones_bf[DH:DH + 1, 0:DH],
                                 l1[DH:DH + 1, :], start=True, stop=True)
                rbb = small.tile([DH, 512], F32, name=f"rb{qt}h{h}i{i}",
                                 tag="rbb")
                nc.vector.reciprocal_approx_fast(rbb[:], bb[0:DH, :])
                nc.vector.tensor_mul(out=CT[0:DH, fcH, q0:q0 + 512],
                                     in0=cps[i][0:DH, :], in1=rbb[:])
                continue
            csb = small.tile([DH, 512], F32, name=f"cs{qt}h{h}i{i}", tag="csb")
            nc.vector.tensor_copy(out=csb[:], in_=cps[i][0:DH, :])
            rd = dpool.tile([1, 512], BF16, name=f"rd{qt}h{h}i{i}", tag="rd")
            nc.sync.dma_start(rd[:], l1[DH:DH + 1, :])
            lbb = small.tile([DH, 512], F32, name=f"lb{qt}h{h}i{i}", tag="lbb")
            nc.gpsimd.dma_start(lbb[:], rd[0].partition_broadcast(DH))
            rbb = small.tile([DH, 512], F32, name=f"rb{qt}h{h}i{i}", tag="rbb")
            nc.vector.reciprocal_approx_fast(rbb[:], lbb[:])
            if p0 == 0:
                nc.vector.tensor_mul(out=CT[0:DH, fcH, q0:q0 + 512],
                                     in0=csb[:], in1=rbb[:])
            else:
                tmp = small.tile([P, 512], BF16, name=f"t{qt}h{h}i{i}", tag="tmp")
                nc.vector.tensor_mul(out=tmp[0:DH, :], in0=csb[:], in1=rbb[:])
                nc.sync.dma_start(CT[DH:2 * DH, fcH, q0:q0 + 512], tmp[0:DH, :])

    # ------------------------------------------------------------------
    # prologue: K proj (all), Q proj qc0+qc1, V chain kc0 — straight-line.
    # ------------------------------------------------------------------
    def run_chain(tname, qc, fc):
        ps = rr_tile(f"p{tname}{qc}{fc}")
        for mm in proj_chain_mms(tname, qc, fc, ps):
            mm()
        proj_drain(tname, qc, fc, ps)

    # K qc0 emitted dc-major across the four fc accumulators: each dc chunk
    # needs only 256KB of wk+xk0, so the PE streams as the DMA lands instead
    # of stalling ~4us for the whole 2MB.
    k0_ps = [rr_tile(f"pk0{fc}") for fc in range(FC)]
    for dc in range(DC):
        for fc in range(FC):
            nc.tensor.matmul(
                k0_ps[fc][:], wk_sb[:, dc, fc * P:(fc + 1) * P],
                xk[0][:, dc, :], start=(dc == 0), stop=(dc == DC - 1))
    for fc in range(FC):
        proj_drain("k", 0, fc, k0_ps[fc])
    x_dma(xk, "k", 2)
    nc.sync.dma_start(wq_sb[:], io["wqT"][:])
    nc.sync.dma_start(bq_sb[:], io["bqc"].rearrange("(fc p) -> p fc", p=P))
    for h in range(4, NH):
        kt_memset(h)

    for qc in range(1, NQC):
        for fc in range(FC):
            run_chain("k", qc, fc)
        if qc == 1:
            x_dma(xk, "k", 3)
            nc.gpsimd.memset(V4[:, :, :, DH:], 0.0)
        if qc == 2:
            x_dma(xq, "q", 0)
            nc.sync.dma_start(wv_sb[:], io["wvT"][:])
            nc.sync.dma_start(
                bv_sb[:], io["bvc"].unsqueeze(0).partition_broadcast(P))
            nc.gpsimd.memset(V4[:, :, :, DH:DH + 1], 1.0)
            nc.gpsimd.memset(ones_bf[:], 1.0)
        if qc == 3:
            x_dma(xq, "q", 1)
    for qc in range(2):
        for fc in range(FC):
            run_chain("q", qc, fc)
        if qc == 0:
            x_dma(xv, "v", 0)
            nc.sync.dma_start(wo_sb[:], io["woT"][:])
    run_chain("v", 0, 0)  # V kc0 before the unit stream begins

    # ------------------------------------------------------------------
    # main stream: clock-simulated emission of S / exp / wedges / PV.
    # ------------------------------------------------------------------
    units = [(qt, h, kc) for qt in range(NQT) for h in HEAD_ORDER
             for kc in range(KC)]
    NU = len(units)

    # chain-start plan: unit -> list of (kind, args). Chains are *created*
    # (psB alloc) at emission time in plan order.
    starts = [[] for _ in range(NU)]
    # V chains: chain kc starts at unit kc-1 (drains one unit before PV(kc)).
    for kc in range(1, KC):
        starts[kc - 1].append(("v", kc))
    # Q qc2/qc3 chains in head positions 1..4 (2 chains each, offsets 1 & 8)
    qplan = [(2, 0), (2, 1), (2, 2), (2, 3), (3, 0), (3, 1), (3, 2), (3, 3)]
    for j, (qc, fc) in enumerate(qplan):
        hpos = 1 + j // 2
        starts[hpos * KC + 1 + 7 * (j % 2)].append(("q", qc, fc))
    # outproj(qt-1): 2 chains per head position in tile qt. The first head's
    # first chain starts late (kc4): it waits on the previous tile's last
    # normalize, whose DRAM-bounce broadcast has ~4us latency.
    for qt in range(1, NQT):
        base = qt * NH * KC
        for hi in range(NH):
            off1 = 4 if hi == 0 else 1
            starts[base + hi * KC + off1].append(("o", qt - 1, hi, 0))
            starts[base + hi * KC + 12].append(("o", qt - 1, hi, 1))

    # wedge steps pending per unit: list of (kind, fn)
    wedge_steps = [[] for _ in range(NU)]

    def plan_chain(u0, mms, drain, per_unit=1):
        u, i = u0, 0
        while i < len(mms):
            for _ in range(per_unit):
                if i < len(mms):
                    wedge_steps[u].append(("mm", mms[i]))
                    i += 1
            u += 1
        wedge_steps[u - 1].append(("drain", drain))

    state = {"pe": 0.0, "sc": 0.0, "stall": 0.0}
    exp_done = {}
    es_ring = {}
    s_idx = [0]
    pv_idx = [0]

    def emit_S():
        si = s_idx[0]
        qt, h, kc = units[si]
        fcH = h // 2
        sps = psA.tile([P, QW], F32, name=f"s{qt}h{h}k{kc}", tag="score")
        for i in range(NI):
            q0 = qt * QW + i * 512
            nc.tensor.matmul(
                sps[:, i * 512:(i + 1) * 512],
                KT[:, h, kc * P:(kc + 1) * P],
                QT[:, fcH, q0:q0 + 512],
                start=True, stop=True)
        state["pe"] += NI * MM_NS
        es = epool.tile([P, QW], BF16, name=f"e{qt}h{h}k{kc}", tag="expS")
        nc.scalar.activation(es[:], sps[:], EXP, scale=SCALE)
        state["sc"] = max(state["sc"], state["pe"]) + EXP_NS
        exp_done[si] = state["sc"]
        es_ring[si] = es
        s_idx[0] += 1

    def pump_S():
        # Emit scores as early as the psA double-buffer allows: ScalarE then
        # runs continuously and banks es tiles ahead of the PV stream, which
        # bridges the stretches where exp/unit exceeds PE work/unit.
        while True:
            si = s_idx[0]
            if si >= NU or si - pv_idx[0] >= BANK_CAP:
                return
            if si >= 2 and exp_done[si - 2] > state["pe"] + PSA_SLACK:
                return  # psA bank would still be busy — emitting gaps PE
            emit_S()

    cps_of_head = {}
    wedge_count = [0]
    for u in range(NU):
        qt, h, kc = units[u]
        ph = u // KC  # global head position
        if kc == 0:
            # PV accumulator pair: pools (0,1) on even head positions,
            # (2,3) on odd — the other pair serves this head's wedges.
            cpools = (0, 1) if ph % 2 == 0 else (2, 3)
            cps_of_head[(qt, h)] = [
                acc_tile(cpools[i], f"c{qt}h{h}i{i}") for i in range(NI)]
            wedge_count[0] = 0
        cps = cps_of_head[(qt, h)]
        wpools = (2, 3) if ph % 2 == 0 else (0, 1)

        # create chains scheduled to start at this unit (PSUM alloc order
        # matches emission order)
        for item in starts[u]:
            wp = wpools[wedge_count[0] % 2]
            wedge_count[0] += 1
            if item[0] == "v":
                kcc = item[1]
                ps = acc_tile(wp, f"pv{kcc}")
                plan_chain(u, proj_chain_mms("v", kcc // 4, kcc % 4, ps),
                           (lambda kcc=kcc, ps=ps:
                            proj_drain("v", kcc // 4, kcc % 4, ps)),
                           per_unit=8)
            elif item[0] == "q":
                _, qcc, fcc = item
                ps = acc_tile(wp, f"pq{qcc}{fcc}")
                plan_chain(u, proj_chain_mms("q", qcc, fcc, ps),
                           (lambda qcc=qcc, fcc=fcc, ps=ps:
                            proj_drain("q", qcc, fcc, ps)))
            else:
                _, pqt, hi2, ci = item
                mms, drain = outproj_chain(
                    pqt, hi2, ci, lambda nm, wp=wp: acc_tile(wp, nm))
                plan_chain(u, mms, drain)

        # x DMA prefetches for upcoming wedge chains (issue a few units
        # ahead of first use; transfers overlap the attention stream)
        if u == 1:
            x_dma(xv, "v", 1)
        if u == 4:
            x_dma(xv, "v", 2)
        if u == 8:
            x_dma(xv, "v", 3)
        if u == 12:
            x_dma(xq, "q", 2)
        if u == 2 * KC + 12:
            x_dma(xq, "q", 3)

        pump_S()
        for step in wedge_steps[u]:
            kind, fn = step
            fn()
            if kind == "mm":
                state["pe"] += MM_NS
                pump_S()

        if __debug__ and DEBUG_SCHED:
            print(f"u={u:3d} pe={state['pe']/1000:8.2f} sc={state['sc']/1000:8.2f} "
                  f"s={s_idx[0]:3d} bank={s_idx[0]-pv_idx[0]:2d} "
                  f"stall={state['stall']/1000:6.2f}", file=sys.stderr)
        # PV for this unit
        while s_idx[0] <= u:
            emit_S()  # safety: es must exist (sim may stall)
        if exp_done[u] > state["pe"]:
            state["stall"] += exp_done[u] - state["pe"]
            state["pe"] = exp_done[u]
        es = es_ring.pop(u)
        for i in range(NI):
            nc.tensor.matmul(
                cps[i][0:SW, :], V[:, kc, h * SW:(h + 1) * SW],
                es[:, i * 512:(i + 1) * 512],
                start=(kc == 0), stop=(kc == KC - 1))
        state["pe"] += NI * MM_NS
        if u < KC:
            # h0 units run ~0.4us slower than the plain sum of their matmuls
            # (DMA/drain interleaving); without this the sim paces scores too
            # late and ScalarE gaps near the end of h0
            state["pe"] += 400.0
        pv_idx[0] = u + 1
        if kc == KC - 1:
            emit_normalize(qt, h, cps, tail=(u == NU - 1))

    # ------------------------------------------------------------------
    # tail: output projection of the last tile (sc 0..3 gated only on the
    # final head's i=0 normalize, which used the fast PE-broadcast path).
    # ------------------------------------------------------------------
    qt = NQT - 1
    rr[0] = 2  # first tail chain lands on the pool freed by the i=0 mul,
    # matching its CT data dependency
    for s8 in range(QW // P):
        for oc in range(2):
            mms, drain = outproj_chain(
                qt, s8, oc, rr_tile, copy_eng="scalar" if oc == 1 else None,
                dma_eng=nc.scalar if oc == 1 else nc.sync)
            for mm in mms:
                mm()
            drain()


def build_program(seq=S, num_devices=8):
    nc = bacc.Bacc("TRN2", target_bir_lowering=False, debug=False,
                   num_devices=num_devices)
    nqc = seq // 512
    io = {
        "xqT": nc.dram_tensor("xqT", (nqc, P, DC * 512), BF16, kind="ExternalInput").ap(),
        "xkT": nc.dram_tensor("xkT", (nqc, P, DC * 512), BF16, kind="ExternalInput").ap(),
        "xvT": nc.dram_tensor("xvT", (nqc, P, DC * 512), BF16, kind="ExternalInput").ap(),
        "wqT": nc.dram_tensor("wqT", (P, DC, G), BF16, kind="ExternalInput").ap(),
        "wkT": nc.dram_tensor("wkT", (P, DC, G), BF16, kind="ExternalInput").ap(),
        "wvT": nc.dram_tensor("wvT", (P, DC, G), BF16, kind="ExternalInput").ap(),
        "woT": nc.dram_tensor("woT", (P, FC, D), BF16, kind="ExternalInput").ap(),
        "bqc": nc.dram_tensor("bqc", (G,), F32, kind="ExternalInput").ap(),
        "bkc": nc.dram_tensor("bkc", (G,), F32, kind="ExternalInput").ap(),
        "bvc": nc.dram_tensor("bvc", (G,), F32, kind="ExternalInput").ap(),
        "out": nc.dram_tensor("out", (seq, D), BF16, kind="ExternalOutput").ap(),
    }
    with tile.TileContext(nc) as tc:
        with ExitStack() as ctx:
            _emit(ctx, tc, io, seq)
    nc.compile()
    return nc


_PROG = None


def _get_prog():
    global _PROG
    if _PROG is None:
        _PROG = build_program()
    return _PROG


def make_in_maps(q, k, v, wq, bq, wk, bk, wv, bv, wo):
    bf16 = ml_dtypes.bfloat16
    f32 = np.float32
    NQC = S // 512

    def xdev(x):
        t = x.T.reshape(DC, P, NQC, 512).transpose(2, 1, 0, 3)
        return np.ascontiguousarray(t).astype(bf16).reshape(NQC, P, DC * 512)

    def wdev(w):
        return np.ascontiguousarray(
            w.T.reshape(DC, P, G).transpose(1, 0, 2)).astype(bf16)

    xT = []
    for b in range(B):
        xT.append((xdev(q[b]), xdev(k[b]), xdev(v[b])))
    halves = []
    for hh in range(2):
        rows = slice(hh * G, (hh + 1) * G)
        halves.append({
            "wqT": wdev(wq[rows, :]),
            "wkT": wdev(wk[rows, :]),
            "wvT": wdev(wv[rows, :]),
            "woT": np.ascontiguousarray(
                wo[:, rows].T.reshape(FC, P, D).transpose(1, 0, 2)).astype(bf16),
            "bqc": np.ascontiguousarray(bq[rows]).astype(f32),
            "bkc": np.ascontiguousarray(bk[rows]).astype(f32),
            "bvc": np.ascontiguousarray(bv[rows]).astype(f32),
        })
    in_maps = []
    for c in range(8):
        b, hh = c // 2, c % 2
        m = dict(halves[hh])
        m["xqT"], m["xkT"], m["xvT"] = xT[b]
        in_maps.append(m)
    return in_maps


def run_with_results(q, k, v, wq, bq, wk, bk, wv, bv, wo, bo, **kw):
    nc = _get_prog()
    in_maps = make_in_maps(np.asarray(q, np.float32), np.asarray(k, np.float32),
                           np.asarray(v, np.float32), np.asarray(wq, np.float32),
                           np.asarray(bq, np.float32), np.asarray(wk, np.float32),
                           np.asarray(bk, np.float32), np.asarray(wv, np.float32),
                           np.asarray(bv, np.float32), np.asarray(wo, np.float32))
    res = run_bass_kernel_spmd(nc, in_maps, core_ids=list(range(8)), **kw)
    parts = [np.asarray(res.results[c]["out"], dtype=np.float32)
             for c in range(8)]
    bo = np.asarray(bo, np.float32)
    out = np.stack([parts[2 * b] + parts[2 * b + 1] + bo for b in range(B)])
    return out.astype(np.float32), res


def kernel(q, k, v, wq, bq, wk, bk, wv, bv, wo, bo):
    out, _ = run_with_results(q, k, v, wq, bq, wk, bk, wv, bv, wo, bo)
    return out
